# revision 1
# baseline (speedup 1.0000x reference)
"""Causal self-attention (B=2, T=2048, C=1024, H=16, D=64) on 8 trn2 cores.

Sharding: tensor-parallel on heads — 2 heads per core. Each core computes
QKV projection for its 2 heads, causal softmax attention, and its heads'
slice of the output projection (a rank-128 partial sum of the full output).
The host pre-transposes x to [B, C, T], slices the weights per core, and
sums the 8 partial outputs (+ proj bias) at the end.

Device kernel layout notes (per core):
  - x^T chunks [128(C), T] stream from DRAM (host-transposed, contiguous
    DMA); QKV computed as W^T @ x^T giving q/k/v in [feat, tok] layout,
    which is what the attention matmuls want. All matmul inputs bf16,
    accumulation fp32.
  - S^T = kT-slice.T @ qT tile -> [128(k), Q] PSUM; exp on ScalarE with
    the 1/sqrt(D) scale fused; causality via subtile skipping, a
    diagonal-aligned exp range, and one [128,128] triangle mask multiply.
  - O^T accumulates as (V|1)-chunk.T @ P^T; the ones column makes row 64
    the softmax denominator for free. Normalization: fast reciprocal,
    GpSimd partition-broadcast, one multiply into a [128, Q] tile with
    both heads stacked.
  - Projection: single K=128 matmul per output tile (heads contracted
    together against the raw [128, C] proj_w slice).
  - The attention inner loop ping-pongs PE<->ACT, which leaves the PE
    sparse and HAM-throttled at 1.2 GHz. Dense independent PE work (next
    batch's QKV, V transposes, previous q-tile's projection) is emitted
    through a filler queue, one item per chunk, to keep the PE streaming.
"""

from collections import deque

import numpy as np

import concourse.bass as bass
import concourse.tile as tile
from concourse import bacc, mybir
from concourse.bass_utils import run_bass_kernel_spmd

dt = mybir.dt
AF = mybir.ActivationFunctionType

B, T, C, H, D = 2, 2048, 1024, 16, 64
NCORES = 8
HPC = H // NCORES          # heads per core = 2
QT = 1024                  # q-tile (columns of S^T/O^T psum tiles)
KC = 128                   # k chunk (partition dim of S^T)
SUB = 512                  # psum bank subtile (fp32)
SCALE = 1.0 / 8.0          # 1/sqrt(D)

_CACHE = {}


def _emit(tc):
    from contextlib import ExitStack
    with ExitStack() as ctx:
        _emit_body(tc, ctx)


def _emit_body(tc, ctx):
    nc = tc.nc
    f32, bf16 = dt.float32, dt.bfloat16

    xT = nc.dram_tensor("xT", [B, C, T], bf16, kind="ExternalInput").ap()
    wqkv = nc.dram_tensor("wqkv", [C, 384], bf16, kind="ExternalInput").ap()
    bqkv = nc.dram_tensor("bqkv", [128, 3], f32, kind="ExternalInput").ap()
    wp = nc.dram_tensor("wp", [128, C], bf16, kind="ExternalInput").ap()
    tri = nc.dram_tensor("tri", [128, 128], bf16, kind="ExternalInput").ap()
    ident = nc.dram_tensor("ident", [128, 128], bf16, kind="ExternalInput").ap()
    outp = nc.dram_tensor("outp", [B, T, C], f32, kind="ExternalOutput").ap()

    consts = ctx.enter_context(tc.tile_pool(name="consts", bufs=1))
    xpool = ctx.enter_context(tc.tile_pool(name="xpool", bufs=2))
    qkvpool = ctx.enter_context(tc.tile_pool(name="qkvpool", bufs=6))
    vtmpool = ctx.enter_context(tc.tile_pool(name="vtmpool", bufs=2))
    ptpool = ctx.enter_context(tc.tile_pool(name="ptpool", bufs=8))
    unormp = ctx.enter_context(tc.tile_pool(name="unormp", bufs=3))
    rows = ctx.enter_context(tc.tile_pool(name="rows", bufs=4))
    outsb = ctx.enter_context(tc.tile_pool(name="outsb", bufs=8))
    stp = ctx.enter_context(tc.tile_pool(name="stp", bufs=2, space="PSUM"))
    otp = ctx.enter_context(tc.tile_pool(name="otp", bufs=1, space="PSUM"))
    miscp = ctx.enter_context(tc.tile_pool(name="miscp", bufs=2, space="PSUM"))

    # constants / weights resident in SBUF
    w_sb = consts.tile([128, 8, 384], bf16, tag="w")
    nc.sync.dma_start(out=w_sb, in_=wqkv.rearrange("(k p) f -> p k f", p=128))
    b_sb = consts.tile([128, 3], f32, tag="b")
    nc.sync.dma_start(out=b_sb, in_=bqkv)
    wp_sb = consts.tile([128, C], bf16, tag="wp")
    nc.sync.dma_start(out=wp_sb, in_=wp)
    tri_sb = consts.tile([128, 128], bf16, tag="tri")
    nc.sync.dma_start(out=tri_sb, in_=tri)
    id_sb = consts.tile([128, 128], bf16, tag="id")
    nc.sync.dma_start(out=id_sb, in_=ident)

    # x^T for both batches (sync queue, ahead of output stores)
    xps = []
    for b in range(B):
        xp = xpool.tile([128, 8, T], bf16, tag="xp", name=f"xp{b}")
        xsrc = xT[b].rearrange("(j p) t -> p j t", p=128)
        for tg in range(T // SUB):
            t0 = tg * SUB
            nc.sync.dma_start(out=xp[:, :, t0:t0 + SUB],
                              in_=xsrc[:, :, t0:t0 + SUB])
        xps.append(xp)

    filler = deque()

    def pop_filler():
        if filler:
            filler.popleft()()

    def make_qkv(b):
        """qkvT tiles + one thunk per (m, token-group): an 8-MM dense chain."""
        dsts = [qkvpool.tile([128, T], bf16, tag="qkv", name=f"qkv{b}_{m}")
                for m in range(3)]
        thunks = []
        for tg in range(T // 1024):
            for m in range(3):
                def th(m=m, tg=tg):
                    pgs = [miscp.tile([128, SUB], f32, tag="misc",
                                      name=f"pg{n}") for n in range(2)]
                    for kc in range(8):
                        for n in range(2):
                            t0 = tg * 1024 + n * SUB
                            nc.tensor.matmul(
                                pgs[n][:, :],
                                w_sb[:, kc, 128 * m:128 * m + 128],
                                xps[b][:, kc, t0:t0 + SUB],
                                start=(kc == 0), stop=(kc == 7),
                            )
                    for n in range(2):
                        t0 = tg * 1024 + n * SUB
                        nc.scalar.activation(
                            dsts[m][:, t0:t0 + SUB], pgs[n][:, :],
                            AF.Identity, bias=b_sb[:, m:m + 1])
                thunks.append(th)
        return dsts, thunks

    def make_vt(b, vT_t):
        """V to token-major [128, 16, 2*65] with ones columns; 9 thunks."""
        vt = vtmpool.tile([128, 16, HPC * 65], bf16, tag="vtm", name=f"vt{b}")

        def th0():
            nc.vector.memset(
                vt.rearrange("p k (h c) -> p k h c", h=HPC)[:, :, :, 64:65],
                1.0)
        thunks = [th0]
        for j0 in range(0, T // 128, 2):
            def th(j0=j0):
                for j in (j0, j0 + 1):
                    tp = miscp.tile([128, 128], bf16, tag="misc", name="tp")
                    nc.tensor.transpose(
                        tp[:, :], vT_t[:, 128 * j:128 * j + 128], id_sb[:, :])
                    nc.vector.tensor_copy(
                        out=vt[:, j, :].rearrange(
                            "p (h c) -> p h c", h=HPC)[:, :, 0:64],
                        in_=tp.rearrange("p (h c) -> p h c", h=HPC),
                    )
            thunks.append(th)
        return vt, thunks

    def make_proj(b, q0, un):
        """Projection of one q-tile: 16 single-matmul thunks."""
        thunks = []
        for ts in range(QT // 128):
            for ct in range(C // SUB):
                def th(ts=ts, ct=ct):
                    a0 = q0 + ts * 128
                    pp = miscp.tile([128, SUB], f32, tag="misc", name="pp")
                    nc.tensor.matmul(
                        pp[:, :],
                        un[:, ts * 128:(ts + 1) * 128],
                        wp_sb[:, ct * SUB:(ct + 1) * SUB],
                        start=True, stop=True,
                    )
                    ob = outsb.tile([128, SUB], f32, tag="osb")
                    nc.vector.tensor_copy(ob[:, :], pp[:, :])
                    nc.sync.dma_start(
                        out=outp[b, a0:a0 + 128, ct * SUB:(ct + 1) * SUB],
                        in_=ob[:, :])
                thunks.append(th)
        return thunks

    # batch 0 front work runs densely right away
    qkv0, th0 = make_qkv(0)
    for th in th0:
        th()
    vt0, vth0 = make_vt(0, qkv0[2])
    for th in vth0:
        th()

    qkv_t, vt_t = {0: qkv0}, {0: vt0}

    for b in range(B):
        if b == 0:
            # queue batch 1 front work as attention filler
            qkv1, th1 = make_qkv(1)
            vt1, vth1 = make_vt(1, qkv1[2])
            filler.extend(th1)
            filler.extend(vth1)
            qkv_t[1], vt_t[1] = qkv1, vt1
        qT_t, kT_t, vT_t = qkv_t[b]
        vt = vt_t[b]

        for qt in range(T // QT):
            q0 = qt * QT
            nkc = (q0 + QT) // KC
            un = unormp.tile([128, QT], bf16, tag="un", name=f"un{b}{qt}")
            for h in range(HPC):
                qT_h = qT_t[64 * h:64 * h + 64, :]
                kT_h = kT_t[64 * h:64 * h + 64, :]
                ot = otp.tile([65, QT], f32, tag="ot")

                def emit_o(kc, pt_):
                    ls = max(0, kc * KC - q0)
                    diag = kc * KC >= q0
                    for n in range(QT // SUB):
                        s0 = max(n * SUB, ls)
                        if s0 >= (n + 1) * SUB:
                            continue
                        if diag and s0 == ls:
                            s0 = ls + 128  # masked strip emitted separately
                            if s0 >= (n + 1) * SUB:
                                continue
                        last_kc = (q0 + (n + 1) * SUB) // KC - 1
                        nc.tensor.matmul(
                            ot[:, s0:(n + 1) * SUB],
                            vt[:, kc, 65 * h:65 * h + 65],
                            pt_[:, s0:(n + 1) * SUB],
                            start=(kc == 0), stop=(kc == last_kc),
                        )
                    if diag:
                        # region already started by kc=0's full-subtile MM
                        n0 = ls // SUB
                        last_kc = (q0 + (n0 + 1) * SUB) // KC - 1
                        nc.tensor.matmul(
                            ot[:, ls:ls + 128],
                            vt[:, kc, 65 * h:65 * h + 65],
                            pt_[:, ls:ls + 128],
                            start=False, stop=(kc == last_kc),
                        )

                for kc in range(nkc):
                    k0 = kc * KC
                    ls = max(0, k0 - q0)
                    st = stp.tile([128, QT], f32, tag="st")
                    pt_ = ptpool.tile([128, QT], bf16, tag="pt")
                    for n in range(QT // SUB):
                        s0 = max(n * SUB, ls)
                        if s0 >= (n + 1) * SUB:
                            continue
                        nc.tensor.matmul(
                            st[:, s0:(n + 1) * SUB],
                            kT_h[:, k0:k0 + KC],
                            qT_h[:, q0 + s0:q0 + (n + 1) * SUB],
                            start=True, stop=True,
                        )
                    nc.scalar.activation(
                        pt_[:, ls:QT], st[:, ls:QT], AF.Exp, scale=SCALE)
                    if k0 >= q0:  # diagonal chunk: zero invalid triangle
                        nc.vector.tensor_mul(
                            pt_[:, ls:ls + 128], pt_[:, ls:ls + 128],
                            tri_sb[:, :])
                    emit_o(kc, pt_)
                    if not (b == 0 and qt == 0 and kc % 2 == 0):
                        pop_filler()

                # normalize into this head's half of un
                se = rows.tile([1, QT], f32, tag="se", name=f"se{h}")
                nc.vector.tensor_copy(se[:, :], ot[64:65, :])
                rc = rows.tile([1, QT], f32, tag="rc", name=f"rc{h}")
                nc.vector.reciprocal_approx_fast(rc[:, :], se[:, :])
                rb = rows.tile([64, QT], f32, tag="rb", name=f"rb{h}")
                nc.gpsimd.partition_broadcast(rb[:, :], rc[:, :])
                nc.vector.tensor_mul(
                    un[64 * h:64 * h + 64, :], ot[0:64, :], rb[:, :])
            filler.extend(make_proj(b, q0, un))

    while filler:
        pop_filler()


def build():
    if "nc" in _CACHE:
        return _CACHE["nc"]
    nc = bacc.Bacc("TRN2", target_bir_lowering=False, debug=False,
                   num_devices=NCORES)
    with tile.TileContext(nc) as tc:
        _emit(tc)
    nc.compile()
    _CACHE["nc"] = nc
    return nc


def make_in_maps(x, qkv_w, qkv_b, proj_w):
    import ml_dtypes
    bf16 = ml_dtypes.bfloat16
    x = np.asarray(x, dtype=np.float32)
    qkv_w = np.asarray(qkv_w, dtype=np.float32)
    qkv_b = np.asarray(qkv_b, dtype=np.float32)
    proj_w = np.asarray(proj_w, dtype=np.float32)

    xT = np.ascontiguousarray(x.transpose(0, 2, 1)).astype(bf16)
    tri = (np.arange(128)[None, :] >= np.arange(128)[:, None]).astype(bf16)
    ident = np.eye(128, dtype=bf16)

    in_maps = []
    for c in range(NCORES):
        s = 64 * HPC * c  # first feature row of this core's heads
        wq = qkv_w[:, s:s + 128]
        wk = qkv_w[:, C + s:C + s + 128]
        wv = qkv_w[:, 2 * C + s:2 * C + s + 128]
        wqkv_c = np.ascontiguousarray(
            np.concatenate([wq, wk, wv], axis=1)).astype(bf16)
        bqkv_c = np.ascontiguousarray(np.stack(
            [qkv_b[s:s + 128], qkv_b[C + s:C + s + 128],
             qkv_b[2 * C + s:2 * C + s + 128]], axis=1))
        wp_c = np.ascontiguousarray(proj_w[s:s + 128, :]).astype(bf16)
        in_maps.append({
            "xT": xT, "wqkv": wqkv_c, "bqkv": bqkv_c, "wp": wp_c,
            "tri": tri, "ident": ident,
        })
    return in_maps


def kernel(x, qkv_w, qkv_b, proj_w, proj_b, _trace=False):
    nc = build()
    in_maps = make_in_maps(x, qkv_w, qkv_b, proj_w)
    res = run_bass_kernel_spmd(nc, in_maps, core_ids=list(range(NCORES)),
                               trace=_trace)
    acc = np.zeros((B, T, C), dtype=np.float64)
    for c in range(NCORES):
        acc += res.results[c]["outp"].astype(np.float64)
    acc += np.asarray(proj_b, dtype=np.float64)
    out = acc.astype(np.float32)
    _CACHE["last_results"] = res
    return out



# revision 3
# speedup vs baseline: 1.1190x; 1.1190x over previous
"""Causal self-attention (B=2, T=2048, C=1024, H=16, D=64) on 8 trn2 cores.

Sharding: tensor-parallel on heads — 2 heads per core. Each core computes
QKV projection for its 2 heads, causal softmax attention, and its heads'
slice of the output projection (a rank-128 partial sum of the full output).
The host pre-transposes x to [B, C, T], slices the weights per core, and
sums the 8 partial outputs (+ proj bias + v-bias@proj_w correction).

Bias algebra (exact):
  - k bias: softmax rows are shift-invariant, so S = (q+bq). k_raw modulo
    per-row constants — the k bias is dropped entirely on device.
  - v bias: rows of normalized attention sum to 1, so attn@(V+1 bv^T) =
    attn@V + 1 bv^T; the bv@proj_w term is a constant row added on host.
  Only the q bias remains on device (fused into the q PSUM->SBUF activation).

Device kernel layout (per core):
  - x^T chunks [128(C), T] stream from DRAM; QKV computed as W^T @ x^T
    giving q/k/v in [feat, tok] layout. All matmul inputs bf16, fp32 accum.
  - Attention q-tiles of 512 tokens. Per k-chunk (128 tokens) the TWO
    heads' S^T matmuls are emitted back-to-back: K=64 contractions at
    base partitions 0/64 auto-derive tile_position (0,0)/(64,0), so they
    run CONCURRENTLY in the two row-halves of the PE array (row tiling).
  - One fused exp ACTIVATE per (q-tile, k-chunk) covers both heads via a
    [128, 2, 512] PSUM pair tile (halves ScalarE instruction overhead).
  - O^T accumulates as (V|1)-chunk.T @ P^T; the ones column makes row 64
    the softmax denominator. Causality: subtile skipping + one [128,128]
    triangle mask multiply per head on the diagonal chunk.
  - Projection: single K=128 matmul per output tile.
  - HAM warm-start: a burst of junk matmuls on the identity tile runs
    during the initial x-DMA wait so real work starts at 2.4 GHz, and a
    filler queue (QKV/V-transpose/projection thunks) keeps the PE dense
    through the ScalarE-paced attention inner loop.
"""

from collections import deque

import numpy as np

import concourse.bass as bass
import concourse.tile as tile
from concourse import bacc, mybir
from concourse.bass_utils import run_bass_kernel_spmd

dt = mybir.dt
AF = mybir.ActivationFunctionType

B, T, C, H, D = 2, 2048, 1024, 16, 64
NCORES = 8
HPC = H // NCORES          # heads per core = 2
QT = 512                   # q-tile (columns of S^T/O^T psum tiles)
KC = 128                   # k chunk (partition dim of S^T)
SCALE = 1.0 / 8.0          # 1/sqrt(D)
NWARM = 40                 # junk matmuls to warm the HAM clock gate

_CACHE = {}


def _emit(tc):
    from contextlib import ExitStack
    with ExitStack() as ctx:
        _emit_body(tc, ctx)


def _emit_body(tc, ctx):
    nc = tc.nc
    f32, bf16 = dt.float32, dt.bfloat16

    xT = nc.dram_tensor("xT", [B, C, T], bf16, kind="ExternalInput").ap()
    wqkv = nc.dram_tensor("wqkv", [C, 384], bf16, kind="ExternalInput").ap()
    bq = nc.dram_tensor("bq", [128, 1], f32, kind="ExternalInput").ap()
    wp = nc.dram_tensor("wp", [128, C], bf16, kind="ExternalInput").ap()
    tri = nc.dram_tensor("tri", [128, 128], bf16, kind="ExternalInput").ap()
    ident = nc.dram_tensor("ident", [128, 128], bf16, kind="ExternalInput").ap()
    outp = nc.dram_tensor("outp", [B, T, C], f32, kind="ExternalOutput").ap()

    consts = ctx.enter_context(tc.tile_pool(name="consts", bufs=1))
    xpool = ctx.enter_context(tc.tile_pool(name="xpool", bufs=2))
    qkvpool = ctx.enter_context(tc.tile_pool(name="qkvpool", bufs=6))
    vtmpool = ctx.enter_context(tc.tile_pool(name="vtmpool", bufs=2))
    ptpool = ctx.enter_context(tc.tile_pool(name="ptpool", bufs=4))
    unormp = ctx.enter_context(tc.tile_pool(name="unormp", bufs=3))
    rows = ctx.enter_context(tc.tile_pool(name="rows", bufs=4))
    outsb = ctx.enter_context(tc.tile_pool(name="outsb", bufs=8))
    stp = ctx.enter_context(tc.tile_pool(name="stp", bufs=2, space="PSUM"))
    otp = ctx.enter_context(tc.tile_pool(name="otp", bufs=2, space="PSUM"))
    miscp = ctx.enter_context(tc.tile_pool(name="miscp", bufs=2, space="PSUM"))

    # constants / weights resident in SBUF (small DMAs land first)
    tri_sb = consts.tile([128, 128], bf16, tag="tri")
    nc.sync.dma_start(out=tri_sb, in_=tri)
    id_sb = consts.tile([128, 128], bf16, tag="id")
    nc.sync.dma_start(out=id_sb, in_=ident)
    w_sb = consts.tile([128, 8, 384], bf16, tag="w")
    nc.sync.dma_start(out=w_sb, in_=wqkv.rearrange("(k p) f -> p k f", p=128))
    bq_sb = consts.tile([128, 1], f32, tag="b")
    nc.sync.dma_start(out=bq_sb, in_=bq)
    wp_sb = consts.tile([128, C], bf16, tag="wp")
    nc.sync.dma_start(out=wp_sb, in_=wp)

    # x^T for both batches, chunked so the first QKV tile only waits ~3us
    xps = []
    for b in range(B):
        xp = xpool.tile([128, 8, T], bf16, tag="xp", name=f"xp{b}")
        xsrc = xT[b].rearrange("(j p) t -> p j t", p=128)
        for tg in range(T // QT):
            t0 = tg * QT
            nc.sync.dma_start(out=xp[:, :, t0:t0 + QT],
                              in_=xsrc[:, :, t0:t0 + QT])
        xps.append(xp)

    # HAM warm-up: junk matmuls during the x-DMA wait
    wu = miscp.tile([128, 128], f32, tag="misc", name="wu")
    for _ in range(NWARM):
        nc.tensor.matmul(wu[:, :], id_sb[:, :], id_sb[:, :],
                         start=True, stop=True)

    # per-batch q/k/v in [feat, tok] layout and V in token-major layout
    qkv_t = {b: [qkvpool.tile([128, T], bf16, tag="qkv", name=f"qkv{b}_{m}")
                 for m in range(3)] for b in range(B)}
    vt_t = {b: vtmpool.tile([128, 16, HPC * 65], bf16, tag="vtm",
                            name=f"vt{b}")
            for b in range(B)}

    def th_qkv(b, tg, m):
        """One QKV chain: 8 accumulating matmuls + PSUM->SBUF move."""
        def th():
            pg = miscp.tile([128, QT], f32, tag="misc", name="pg")
            t0 = tg * QT
            for kcw in range(8):
                nc.tensor.matmul(
                    pg[:, :],
                    w_sb[:, kcw, 128 * m:128 * m + 128],
                    xps[b][:, kcw, t0:t0 + QT],
                    start=(kcw == 0), stop=(kcw == 7),
                )
            dst = qkv_t[b][m]
            if m == 0:  # q: fused bias
                nc.scalar.activation(dst[:, t0:t0 + QT], pg[:, :],
                                     AF.Identity, bias=bq_sb[:, :])
            else:       # k, v: biases dropped (host-folded)
                nc.vector.tensor_copy(out=dst[:, t0:t0 + QT], in_=pg[:, :])
        return th

    def th_ones(b):
        def th():
            nc.vector.memset(
                vt_t[b].rearrange("p k (h c) -> p k h c", h=HPC)[:, :, :, 64:65],
                1.0)
        return th

    def th_vt(b, j0, nj):
        """V [feat,tok] -> token-major [128, j, (h 65)] via PE transposes."""
        def th():
            vT_t, vt = qkv_t[b][2], vt_t[b]
            for j in range(j0, j0 + nj):
                tp = miscp.tile([128, 128], bf16, tag="misc", name="tp")
                nc.tensor.transpose(
                    tp[:, :], vT_t[:, 128 * j:128 * j + 128], id_sb[:, :])
                nc.vector.tensor_copy(
                    out=vt[:, j, :].rearrange(
                        "p (h c) -> p h c", h=HPC)[:, :, 0:64],
                    in_=tp.rearrange("p (h c) -> p h c", h=HPC),
                )
        return th

    def th_proj(b, q0, un, ts, ct):
        """Projection of one [128 tok, 512 feat] output tile + store."""
        def th():
            a0 = q0 + ts * 128
            pp = miscp.tile([128, 512], f32, tag="misc", name="pp")
            nc.tensor.matmul(
                pp[:, :],
                un[:, ts * 128:(ts + 1) * 128],
                wp_sb[:, ct * 512:(ct + 1) * 512],
                start=True, stop=True,
            )
            ob = outsb.tile([128, 512], f32, tag="osb")
            nc.vector.tensor_copy(out=ob[:, :], in_=pp[:, :])
            nc.sync.dma_start(
                out=outp[b, a0:a0 + 128, ct * 512:(ct + 1) * 512],
                in_=ob[:, :])
        return th

    # front fillers carry (b, tg) tags so attention can force-drain deps
    front = deque()
    projq = deque()
    for b in range(B):
        tgs = range(1, 4) if b == 0 else range(4)
        if b == 1:
            front.append((1, 0, th_ones(1)))
        for tg in tgs:
            front.append((b, tg, th_qkv(b, tg, 1)))   # k
            front.append((b, tg, th_qkv(b, tg, 0)))   # q
            front.append((b, tg, th_qkv(b, tg, 2)))   # v
            front.append((b, tg, th_vt(b, tg * 4, 2)))
            front.append((b, tg, th_vt(b, tg * 4 + 2, 2)))

    slots_left = [2 * sum(4 * (qt + 1) for qt in range(4))]  # 80 kc slots

    def pop_filler():
        if front:
            front.popleft()[2]()
        elif projq:
            projq.popleft()()

    def force_front(b, qt):
        while front and (front[0][0] < b or
                         (front[0][0] == b and front[0][1] <= qt)):
            front.popleft()[2]()

    # batch 0 / tile-group 0 front work runs densely right away
    th_ones(0)()
    th_qkv(0, 0, 1)(); th_qkv(0, 0, 0)(); th_qkv(0, 0, 2)()
    th_vt(0, 0, 2)(); th_vt(0, 2, 2)()

    for b in range(B):
        qT_t, kT_t, vT_t = qkv_t[b]
        vt = vt_t[b]
        for qt in range(T // QT):
            q0 = qt * QT
            nkc = (q0 + QT) // KC
            force_front(b, qt)
            ots = [otp.tile([65, QT], f32, tag="ot", name=f"ot{h}")
                   for h in range(HPC)]
            for kc in range(nkc):
                k0 = kc * KC
                ls = max(0, k0 - q0)
                diag = k0 >= q0
                st = stp.tile([128, HPC, QT], f32, tag="st")
                for h in range(HPC):
                    nc.tensor.matmul(
                        st[:, h, ls:QT],
                        kT_t[64 * h:64 * h + 64, k0:k0 + KC],
                        qT_t[64 * h:64 * h + 64, q0 + ls:q0 + QT],
                        start=True, stop=True,
                    )
                pt = ptpool.tile([128, HPC, QT], bf16, tag="pt")
                nc.scalar.activation(pt[:, :, ls:QT], st[:, :, ls:QT],
                                     AF.Exp, scale=SCALE)
                if diag:
                    for h in range(HPC):
                        nc.vector.tensor_mul(
                            pt[:, h, ls:ls + 128], pt[:, h, ls:ls + 128],
                            tri_sb[:, :])
                # start=True only on the first MM into each ot bank (it
                # clears the whole bank's has_written bits); stop=True only
                # on the last emitted MM (kc=nkc-1's diagonal strip).
                for h in range(HPC):
                    vch = vt[:, kc, 65 * h:65 * h + 65]
                    if diag:
                        nc.tensor.matmul(
                            ots[h][:, ls:ls + 128], vch,
                            pt[:, h, ls:ls + 128],
                            start=(kc == 0), stop=(kc == nkc - 1),
                        )
                        if ls + 128 < QT:
                            nc.tensor.matmul(
                                ots[h][:, ls + 128:QT], vch,
                                pt[:, h, ls + 128:QT],
                                start=False, stop=False,
                            )
                    else:
                        nc.tensor.matmul(
                            ots[h][:, 0:QT], vch, pt[:, h, 0:QT],
                            start=(kc == 0), stop=False,
                        )
                slots_left[0] -= 1
                npop = 1
                if len(front) + len(projq) > slots_left[0]:
                    npop = 2
                if len(front) + len(projq) > 2 * slots_left[0]:
                    npop = 3
                for _ in range(npop):
                    pop_filler()

            # normalize into un: both heads stacked [128 feat, 512 tok]
            un = unormp.tile([128, QT], bf16, tag="un", name=f"un{b}{qt}")
            for h in range(HPC):
                se = rows.tile([1, QT], f32, tag="se", name=f"se{h}")
                nc.vector.tensor_copy(out=se[:, :], in_=ots[h][64:65, :])
                rc = rows.tile([1, QT], f32, tag="rc", name=f"rc{h}")
                nc.vector.reciprocal_approx_fast(rc[:, :], se[:, :])
                rb = rows.tile([64, QT], f32, tag="rb", name=f"rb{h}")
                nc.gpsimd.partition_broadcast(rb[:, :], rc[:, :])
                nc.vector.tensor_mul(
                    un[64 * h:64 * h + 64, :], ots[h][0:64, :], rb[:, :])
            for ts in range(QT // 128):
                for ct in range(2):
                    projq.append(th_proj(b, q0, un, ts, ct))

    while front or projq:
        pop_filler()


def build():
    if "nc" in _CACHE:
        return _CACHE["nc"]
    nc = bacc.Bacc("TRN2", target_bir_lowering=False, debug=False,
                   num_devices=NCORES)
    with tile.TileContext(nc) as tc:
        _emit(tc)
    nc.compile()
    _CACHE["nc"] = nc
    return nc


def make_in_maps(x, qkv_w, qkv_b, proj_w):
    import ml_dtypes
    bf16 = ml_dtypes.bfloat16
    x = np.asarray(x, dtype=np.float32)
    qkv_w = np.asarray(qkv_w, dtype=np.float32)
    qkv_b = np.asarray(qkv_b, dtype=np.float32)
    proj_w = np.asarray(proj_w, dtype=np.float32)

    xT = np.ascontiguousarray(x.transpose(0, 2, 1)).astype(bf16)
    tri = (np.arange(128)[None, :] >= np.arange(128)[:, None]).astype(bf16)
    ident = np.eye(128, dtype=bf16)

    in_maps = []
    for c in range(NCORES):
        s = 64 * HPC * c  # first feature row of this core's heads
        wq = qkv_w[:, s:s + 128]
        wk = qkv_w[:, C + s:C + s + 128]
        wv = qkv_w[:, 2 * C + s:2 * C + s + 128]
        wqkv_c = np.ascontiguousarray(
            np.concatenate([wq, wk, wv], axis=1)).astype(bf16)
        bq_c = np.ascontiguousarray(
            qkv_b[s:s + 128].reshape(128, 1).astype(np.float32))
        wp_c = np.ascontiguousarray(proj_w[s:s + 128, :]).astype(bf16)
        in_maps.append({
            "xT": xT, "wqkv": wqkv_c, "bq": bq_c, "wp": wp_c,
            "tri": tri, "ident": ident,
        })
    return in_maps


def kernel(x, qkv_w, qkv_b, proj_w, proj_b, _trace=False):
    nc = build()
    in_maps = make_in_maps(x, qkv_w, qkv_b, proj_w)
    res = run_bass_kernel_spmd(nc, in_maps, core_ids=list(range(NCORES)),
                               trace=_trace)
    acc = np.zeros((B, T, C), dtype=np.float64)
    for c in range(NCORES):
        acc += res.results[c]["outp"].astype(np.float64)
    # host-folded bias terms: proj bias + v-bias @ proj_w (exact)
    bv = np.asarray(qkv_b, dtype=np.float64)[2 * C:]
    acc += bv @ np.asarray(proj_w, dtype=np.float64)
    acc += np.asarray(proj_b, dtype=np.float64)
    out = acc.astype(np.float32)
    _CACHE["last_results"] = res
    return out
